# revision 1
# baseline (speedup 1.0000x reference)
"""Gaussian distance loss kernel for 8 Trainium2 NeuronCores.

reference math (per term):
    f[i,j,d] = exp(-0.5*(mu1[i,d]-mu2[j,d])^2 / (v1[i,d]+v2[j,d])) / sqrt(v1+v2)
    term = mean(f)
    out  = vaa + vbb - 2*vab

Sharding: rows i split across 8 cores (128 rows each). Each core holds the
full transposed operand tensors [128(d), 1024(j)] so the per-row values
mu1[i,:], v1[i,:] become per-partition scalar columns — broadcast natively by
tensor_scalar/STT ops and the activation bias port. No broadcast copies.

Per row i (free dim = j, 1024 wide):
    lv = Ln(cv + rv_i)            ACT (bias port does the add)
    t  = Exp(-0.5*lv)             ACT (= rsqrt(v), same table set as Ln)
    y  = (cm - rm_i) * t          DVE scalar_tensor_tensor
    y2 = y*y                      DVE tensor_tensor
    e  = Exp(-0.5*y2)             ACT
    acc[:, col] = sum_j(e*t)      DVE tensor_tensor_reduce
Host sums the [128, 384] partials from all 8 cores in float64.

All inputs are packed into one [128, 4608] array so input staging is a single
DMA (one semaphore) — avoids walrus "too many sync wait commands".
"""

import sys

for _p in ("/opt/trn_rl_repo", "/root/.axon_site/_ro/trn_rl_repo"):
    if _p not in sys.path:
        sys.path.append(_p)

import numpy as np

N = 1024
D = 128
NCORES = 8
ROWS = N // NCORES  # 128 rows per core

# packed input column offsets
O_MA, O_MB, O_LVA, O_LVB = 0, N, 2 * N, 3 * N
O_MAR, O_MBR, O_LVAR, O_LVBR = 4 * N, 4 * N + ROWS, 4 * N + 2 * ROWS, 4 * N + 3 * ROWS
PACKED_W = 4 * N + 4 * ROWS


def build_program(repeat=1):
    import concourse.bacc as bacc
    import concourse.mybir as mybir
    import concourse.tile as tile
    from concourse.alu_op_type import AluOpType

    # The act-table placement pass picks table sets greedily, alternating
    # between the exp-only and ln-only sets here — 385 table reloads (one per
    # row, ~0.5ms each on HW = ~196ms total). All our functions (Exp, Ln,
    # Square) live together in natural_log_exp_and_others; blank the other
    # sets (preserving list indices, which are the act_func_set ids) so the
    # pass must use it: one load for the whole program.
    from concourse import hw_specs as _hw

    _orig_gat = _hw.get_activation_tables.__wrapped__

    def _gat_combined(arch):
        t = dict(_orig_gat(arch))
        return {
            k: (v if k == "natural_log_exp_and_others" else set())
            for k, v in t.items()
        }

    bacc.get_activation_tables = _gat_combined

    f32 = mybir.dt.float32
    Act = mybir.ActivationFunctionType

    nc = bacc.Bacc("TRN2", target_bir_lowering=False, debug=False)

    inp_d = nc.dram_tensor("inp", [D, PACKED_W], f32, kind="ExternalInput")
    acc_out = nc.dram_tensor("acc", [D, 3 * ROWS], f32, kind="ExternalOutput")

    with tile.TileContext(nc) as tc:
        with (
            tc.tile_pool(name="inputs", bufs=1) as inp,
            tc.tile_pool(name="accp", bufs=1) as accp,
            tc.tile_pool(name="lv", bufs=2) as lvp,
            tc.tile_pool(name="t", bufs=3) as tp,
            tc.tile_pool(name="y", bufs=2) as yp,
            tc.tile_pool(name="y2", bufs=2) as y2p,
            tc.tile_pool(name="e", bufs=2) as ep,
            tc.tile_pool(name="scr", bufs=2) as scrp,
        ):
            big = inp.tile([D, PACKED_W], f32, tag="big")
            nc.sync.dma_start(big[:], inp_d[:])

            cm_a = big[:, O_MA : O_MA + N]
            cm_b = big[:, O_MB : O_MB + N]
            rm_a = big[:, O_MAR : O_MAR + ROWS]
            rm_b = big[:, O_MBR : O_MBR + ROWS]

            cv_a = inp.tile([D, N], f32, tag="cv_a")
            cv_b = inp.tile([D, N], f32, tag="cv_b")
            rv_a = inp.tile([D, ROWS], f32, tag="rv_a")
            rv_b = inp.tile([D, ROWS], f32, tag="rv_b")
            nc.scalar.activation(cv_a[:], big[:, O_LVA : O_LVA + N], Act.Exp)
            nc.scalar.activation(cv_b[:], big[:, O_LVB : O_LVB + N], Act.Exp)
            nc.scalar.activation(rv_a[:], big[:, O_LVAR : O_LVAR + ROWS], Act.Exp)
            nc.scalar.activation(rv_b[:], big[:, O_LVBR : O_LVBR + ROWS], Act.Exp)

            acc = accp.tile([D, 3 * ROWS], f32, tag="acc")

            terms = [
                (cm_a, cv_a, rm_a, rv_a),  # vaa
                (cm_b, cv_b, rm_a, rv_a),  # vab
                (cm_b, cv_b, rm_b, rv_b),  # vbb
            ] * repeat

            for ti, (cm, cv, rm, rv) in enumerate(terms):
                for i in range(ROWS):
                    col = (ti % 3) * ROWS + i  # repeats overwrite, same result
                    lv = lvp.tile([D, N], f32, tag="lv")
                    nc.scalar.activation(
                        lv[:], cv[:], Act.Ln, bias=rv[:, i : i + 1], scale=1.0
                    )
                    t = tp.tile([D, N], f32, tag="t")
                    nc.scalar.activation(t[:], lv[:], Act.Exp, scale=-0.5)
                    y = yp.tile([D, N], f32, tag="y")
                    nc.vector.scalar_tensor_tensor(
                        y[:],
                        cm,
                        rm[:, i : i + 1],
                        t[:],
                        AluOpType.subtract,
                        AluOpType.mult,
                    )
                    y2 = y2p.tile([D, N], f32, tag="y2")
                    # y*y on GPSIMD: DVE is the critical engine, GPSIMD is idle
                    nc.gpsimd.tensor_tensor(y2[:], y[:], y[:], AluOpType.mult)
                    e = ep.tile([D, N], f32, tag="e")
                    nc.scalar.activation(e[:], y2[:], Act.Exp, scale=-0.5)
                    scr = scrp.tile([D, N], f32, tag="scr")
                    # tensor_tensor_reduce crashes TRN2 at runtime; STT with
                    # accum_out does the same multiply+sum in one DVE pass.
                    nc.vector.scalar_tensor_tensor(
                        scr[:],
                        e[:],
                        1.0,
                        t[:],
                        AluOpType.mult,
                        AluOpType.mult,
                        accum_out=acc[:, col : col + 1],
                    )

            nc.sync.dma_start(acc_out[:], acc[:])

    nc.compile()
    return nc


_PROGRAM_CACHE = {}


def _get_program(repeat=1):
    if repeat not in _PROGRAM_CACHE:
        _PROGRAM_CACHE[repeat] = build_program(repeat)
    return _PROGRAM_CACHE[repeat]


def pack_inputs(mu_a, logvar_a, mu_b, logvar_b):
    ma_t = np.ascontiguousarray(mu_a.T.astype(np.float32))
    mb_t = np.ascontiguousarray(mu_b.T.astype(np.float32))
    lva_t = np.ascontiguousarray(logvar_a.T.astype(np.float32))
    lvb_t = np.ascontiguousarray(logvar_b.T.astype(np.float32))
    in_maps = []
    for c in range(NCORES):
        r0, r1 = c * ROWS, (c + 1) * ROWS
        packed = np.empty((D, PACKED_W), dtype=np.float32)
        packed[:, O_MA : O_MA + N] = ma_t
        packed[:, O_MB : O_MB + N] = mb_t
        packed[:, O_LVA : O_LVA + N] = lva_t
        packed[:, O_LVB : O_LVB + N] = lvb_t
        packed[:, O_MAR : O_MAR + ROWS] = ma_t[:, r0:r1]
        packed[:, O_MBR : O_MBR + ROWS] = mb_t[:, r0:r1]
        packed[:, O_LVAR : O_LVAR + ROWS] = lva_t[:, r0:r1]
        packed[:, O_LVBR : O_LVBR + ROWS] = lvb_t[:, r0:r1]
        in_maps.append({"inp": packed})
    return in_maps


def run_device(mu_a, logvar_a, mu_b, logvar_b, trace=False, repeat=1):
    from concourse.bass_utils import run_bass_kernel_spmd

    nc = _get_program(repeat)
    in_maps = pack_inputs(mu_a, logvar_a, mu_b, logvar_b)
    return run_bass_kernel_spmd(nc, in_maps, list(range(NCORES)), trace=trace)


def reduce_host(results):
    saa = sab = sbb = 0.0
    for r in results:
        acc = np.asarray(r["acc"], dtype=np.float64)
        saa += acc[:, 0:ROWS].sum()
        sab += acc[:, ROWS : 2 * ROWS].sum()
        sbb += acc[:, 2 * ROWS : 3 * ROWS].sum()
    denom = float(N) * N * D
    return np.float32((saa + sbb - 2.0 * sab) / denom)


def kernel(mu_a, logvar_a, mu_b, logvar_b):
    res = run_device(mu_a, logvar_a, mu_b, logvar_b, trace=False)
    return reduce_host(res.results)



# revision 2
# speedup vs baseline: 1.3423x; 1.3423x over previous
"""Gaussian distance loss kernel v3: v2b + i<->j symmetry for vaa/vbb.

reference math (per term):
    f[i,j,d] = exp(-0.5*(mu1[i,d]-mu2[j,d])^2 / (v1[i,d]+v2[j,d])) / sqrt(v1+v2)
    out = mean(faa) + mean(fbb) - 2*mean(fab)

f(i,j)=f(j,i) for the aa/bb terms, so with the wrapped half-band
B(i) = { (i+d) % N : d in [0, N/2) }:
    S_full = 2*S_band - S_diag + S_half
      S_band = sum_i sum_{d=0}^{N/2-1} f(i, (i+d)%N)
      S_diag = sum_i f(i,i)
      S_half = sum_i f(i, (i+N/2)%N)
The ab term has no symmetry and is computed over the full N columns.

SPMD trick: each core's copy of the column tensors is pre-rotated on the
host by its row base (core c, local row k -> global i = 128c + k), so the
band for local row k is always columns [k, k+512) of the rotated array —
identical program on every core, different data.

Layout/pipeline identical to v2b: partition axis = d, 4-row blocks,
bf16 tiles, DVE tensor_scalar row ops, ACT Ln/Exp, Exp-accum finisher,
software-pipelined emission (z one block late, accum two blocks late).
"""

import sys

for _p in ("/opt/trn_rl_repo", "/root/.axon_site/_ro/trn_rl_repo"):
    if _p not in sys.path:
        sys.path.append(_p)

import numpy as np

N = 1024
D = 128
NCORES = 8
ROWS = N // NCORES  # 128 rows per core
R = 4  # rows per block
NBLK = ROWS // R  # 32 blocks per term
H = N // 2  # 512 band width

WA = ROWS + H  # 640: rotated a-array width (band + half reach)
WB = ROWS + N  # 1152: rotated b-array width (full vab reach)

# packed input column offsets (f32 staging)
O_MA = 0
O_MB = O_MA + WA
O_LVA = O_MB + WB
O_LVB = O_LVA + WA
O_MAR = O_LVB + WB
O_MBR = O_MAR + ROWS
O_LVAR = O_MBR + ROWS
O_LVBR = O_LVAR + ROWS
PACKED_W = O_LVBR + ROWS  # 2*WA + 2*WB + 4*ROWS = 4096


def build_program(repeat=1):
    import concourse.bacc as bacc
    import concourse.mybir as mybir
    import concourse.tile as tile
    from concourse.alu_op_type import AluOpType

    from concourse import hw_specs as _hw

    _orig_gat = _hw.get_activation_tables.__wrapped__

    def _gat_combined(arch):
        t = dict(_orig_gat(arch))
        return {
            k: (v if k == "natural_log_exp_and_others" else set())
            for k, v in t.items()
        }

    bacc.get_activation_tables = _gat_combined

    f32 = mybir.dt.float32
    bf16 = mybir.dt.bfloat16
    Act = mybir.ActivationFunctionType

    nc = bacc.Bacc("TRN2", target_bir_lowering=False, debug=False)

    inp_d = nc.dram_tensor("inp", [D, PACKED_W], f32, kind="ExternalInput")
    # acc columns: 32 aa-band | 32 ab | 32 bb-band | diag aa | diag bb
    #              | half aa | half bb
    NACC = 3 * NBLK + 4
    acc_out = nc.dram_tensor("acc", [D, NACC], f32, kind="ExternalOutput")

    with tile.TileContext(nc) as tc:
        with (
            tc.tile_pool(name="inputs", bufs=1) as inp,
            tc.tile_pool(name="accp", bufs=1) as accp,
            tc.tile_pool(name="vs", bufs=2) as vsp,
            tc.tile_pool(name="lv", bufs=3) as lvp,
            tc.tile_pool(name="t", bufs=2) as tp,
            tc.tile_pool(name="dm", bufs=2) as dmp,
            tc.tile_pool(name="y", bufs=2) as yp,
            tc.tile_pool(name="y2", bufs=3) as y2p,
            tc.tile_pool(name="z", bufs=3) as zp,
            tc.tile_pool(name="sink", bufs=2) as sinkp,
            tc.tile_pool(name="smol", bufs=4) as smol,
        ):
            big = inp.tile([D, PACKED_W], f32, tag="big")
            nc.sync.dma_start(big[:], inp_d[:])

            cm_a = inp.tile([D, WA], bf16, tag="cm_a")
            cm_b = inp.tile([D, WB], bf16, tag="cm_b")
            cv_a = inp.tile([D, WA], bf16, tag="cv_a")
            cv_b = inp.tile([D, WB], bf16, tag="cv_b")
            nc.vector.tensor_copy(cm_a[:], big[:, O_MA : O_MA + WA])
            nc.vector.tensor_copy(cm_b[:], big[:, O_MB : O_MB + WB])
            nc.scalar.activation(cv_a[:], big[:, O_LVA : O_LVA + WA], Act.Exp)
            nc.scalar.activation(cv_b[:], big[:, O_LVB : O_LVB + WB], Act.Exp)

            rm_a = big[:, O_MAR : O_MAR + ROWS]
            rm_b = big[:, O_MBR : O_MBR + ROWS]
            rv_a = inp.tile([D, ROWS], f32, tag="rv_a")
            rv_b = inp.tile([D, ROWS], f32, tag="rv_b")
            nc.scalar.activation(rv_a[:], big[:, O_LVAR : O_LVAR + ROWS], Act.Exp)
            nc.scalar.activation(rv_b[:], big[:, O_LVBR : O_LVBR + ROWS], Act.Exp)

            acc = accp.tile([D, NACC], f32, tag="acc")

            # (col, cm, cv, rm, rv, block_idx, row_width)
            blocks = []
            for rep in range(repeat):
                for ti, (cm, cv, rm, rv, w) in enumerate(
                    [
                        (cm_a, cv_a, rm_a, rv_a, H),  # vaa band
                        (cm_b, cv_b, rm_a, rv_a, N),  # vab full
                        (cm_b, cv_b, rm_b, rv_b, H),  # vbb band
                    ]
                ):
                    for b in range(NBLK):
                        blocks.append((ti * NBLK + b, cm, cv, rm, rv, b, w))

            n_total = len(blocks)
            mids = [None] * n_total
            zs = [None] * n_total

            for idx in range(n_total + 2):
                if idx < n_total:
                    col, cm, cv, rm, rv, b, w = blocks[idx]
                    L = R * w
                    vs = vsp.tile([D, R * N], bf16, tag="vs")
                    dm = dmp.tile([D, R * N], bf16, tag="dm")
                    for k in range(R):
                        i = b * R + k
                        j0 = i if w == H else 0
                        nc.vector.tensor_scalar(
                            vs[:, k * w : (k + 1) * w],
                            cv[:, j0 : j0 + w],
                            rv[:, i : i + 1],
                            None,
                            AluOpType.add,
                        )
                        nc.vector.tensor_scalar(
                            dm[:, k * w : (k + 1) * w],
                            cm[:, j0 : j0 + w],
                            rm[:, i : i + 1],
                            None,
                            AluOpType.subtract,
                        )
                    lv = lvp.tile([D, R * N], bf16, tag="lv")
                    nc.scalar.activation(lv[:, :L], vs[:, :L], Act.Ln)
                    t = tp.tile([D, R * N], bf16, tag="t")
                    nc.scalar.activation(t[:, :L], lv[:, :L], Act.Exp, scale=-0.5)
                    y = yp.tile([D, R * N], bf16, tag="y")
                    nc.vector.tensor_tensor(
                        y[:, :L], dm[:, :L], t[:, :L], AluOpType.mult
                    )
                    y2 = y2p.tile([D, R * N], bf16, tag="y2")
                    nc.vector.tensor_tensor(
                        y2[:, :L], y[:, :L], y[:, :L], AluOpType.mult
                    )
                    mids[idx] = (y2, lv, L)
                if idx >= 1 and idx - 1 < n_total:
                    y2_, lv_, L_ = mids[idx - 1]
                    z = zp.tile([D, R * N], bf16, tag="z")
                    nc.vector.tensor_tensor(
                        z[:, :L_], y2_[:, :L_], lv_[:, :L_], AluOpType.add
                    )
                    zs[idx - 1] = (z, L_)
                    mids[idx - 1] = None
                if idx >= 2:
                    zcol = blocks[idx - 2][0]
                    z_, L_ = zs[idx - 2]
                    sink = sinkp.tile([D, R * N], bf16, tag="sink")
                    nc.scalar.activation(
                        sink[:, :L_],
                        z_[:, :L_],
                        Act.Exp,
                        scale=-0.5,
                        accum_out=acc[:, zcol : zcol + 1],
                    )
                    zs[idx - 2] = None

            # diag: f(i,i) = exp(-0.5*ln(2*v_i)) summed over the core's rows
            # (inside the repeat loop so the repeat-differential timing
            # reflects the full per-pass cost; repeats overwrite, same result)
            for di, rv in enumerate([rv_a, rv_b] * repeat):
                di %= 2
                vd = smol.tile([D, ROWS], bf16, tag="vd")
                nc.vector.tensor_scalar(vd[:], rv[:], 2.0, None, AluOpType.mult)
                ld = smol.tile([D, ROWS], bf16, tag="ld")
                nc.scalar.activation(ld[:], vd[:], Act.Ln)
                sd = smol.tile([D, ROWS], bf16, tag="sd")
                nc.scalar.activation(
                    sd[:],
                    ld[:],
                    Act.Exp,
                    scale=-0.5,
                    accum_out=acc[:, 3 * NBLK + di : 3 * NBLK + di + 1],
                )

            # half: f(i, i+H) — elementwise over the core's rows, columns at
            # rotated offset k+H
            for hi, (cm, cv, rm, rv) in enumerate(
                [(cm_a, cv_a, rm_a, rv_a), (cm_b, cv_b, rm_b, rv_b)] * repeat
            ):
                hi %= 2
                vh = smol.tile([D, ROWS], bf16, tag="vh")
                nc.vector.tensor_tensor(
                    vh[:], rv[:], cv[:, H : H + ROWS], AluOpType.add
                )
                lh = smol.tile([D, ROWS], bf16, tag="lh")
                nc.scalar.activation(lh[:], vh[:], Act.Ln)
                th = smol.tile([D, ROWS], bf16, tag="th")
                nc.scalar.activation(th[:], lh[:], Act.Exp, scale=-0.5)
                dh = smol.tile([D, ROWS], bf16, tag="dh")
                nc.vector.tensor_tensor(
                    dh[:], rm[:], cm[:, H : H + ROWS], AluOpType.subtract
                )
                yh = smol.tile([D, ROWS], bf16, tag="yh")
                nc.vector.tensor_tensor(yh[:], dh[:], th[:], AluOpType.mult)
                y2h = smol.tile([D, ROWS], bf16, tag="y2h")
                nc.vector.tensor_tensor(y2h[:], yh[:], yh[:], AluOpType.mult)
                zh = smol.tile([D, ROWS], bf16, tag="zh")
                nc.vector.tensor_tensor(zh[:], y2h[:], lh[:], AluOpType.add)
                sh = smol.tile([D, ROWS], bf16, tag="sh")
                nc.scalar.activation(
                    sh[:],
                    zh[:],
                    Act.Exp,
                    scale=-0.5,
                    accum_out=acc[:, 3 * NBLK + 2 + hi : 3 * NBLK + 3 + hi],
                )

            nc.sync.dma_start(acc_out[:], acc[:])

    nc.compile()
    return nc


_PROGRAM_CACHE = {}


def _get_program(repeat=1):
    if repeat not in _PROGRAM_CACHE:
        _PROGRAM_CACHE[repeat] = build_program(repeat)
    return _PROGRAM_CACHE[repeat]


def pack_inputs(mu_a, logvar_a, mu_b, logvar_b):
    ma_t = np.ascontiguousarray(mu_a.T.astype(np.float32))
    mb_t = np.ascontiguousarray(mu_b.T.astype(np.float32))
    lva_t = np.ascontiguousarray(logvar_a.T.astype(np.float32))
    lvb_t = np.ascontiguousarray(logvar_b.T.astype(np.float32))

    def rot(x, base, width):
        idx = (base + np.arange(width)) % N
        return x[:, idx]

    in_maps = []
    for c in range(NCORES):
        r0, r1 = c * ROWS, (c + 1) * ROWS
        packed = np.empty((D, PACKED_W), dtype=np.float32)
        packed[:, O_MA : O_MA + WA] = rot(ma_t, r0, WA)
        packed[:, O_MB : O_MB + WB] = rot(mb_t, r0, WB)
        packed[:, O_LVA : O_LVA + WA] = rot(lva_t, r0, WA)
        packed[:, O_LVB : O_LVB + WB] = rot(lvb_t, r0, WB)
        packed[:, O_MAR : O_MAR + ROWS] = ma_t[:, r0:r1]
        packed[:, O_MBR : O_MBR + ROWS] = mb_t[:, r0:r1]
        packed[:, O_LVAR : O_LVAR + ROWS] = lva_t[:, r0:r1]
        packed[:, O_LVBR : O_LVBR + ROWS] = lvb_t[:, r0:r1]
        in_maps.append({"inp": packed})
    return in_maps


def run_device(mu_a, logvar_a, mu_b, logvar_b, trace=False, repeat=1):
    from concourse.bass_utils import run_bass_kernel_spmd

    nc = _get_program(repeat)
    in_maps = pack_inputs(mu_a, logvar_a, mu_b, logvar_b)
    return run_bass_kernel_spmd(nc, in_maps, list(range(NCORES)), trace=trace)


def reduce_host(results):
    band_aa = band_ab = band_bb = 0.0
    diag_aa = diag_bb = half_aa = half_bb = 0.0
    for r in results:
        acc = np.asarray(r["acc"], dtype=np.float64)
        band_aa += acc[:, 0:NBLK].sum()
        band_ab += acc[:, NBLK : 2 * NBLK].sum()
        band_bb += acc[:, 2 * NBLK : 3 * NBLK].sum()
        diag_aa += acc[:, 3 * NBLK].sum()
        diag_bb += acc[:, 3 * NBLK + 1].sum()
        half_aa += acc[:, 3 * NBLK + 2].sum()
        half_bb += acc[:, 3 * NBLK + 3].sum()
    saa = 2.0 * band_aa - diag_aa + half_aa
    sbb = 2.0 * band_bb - diag_bb + half_bb
    sab = band_ab
    denom = float(N) * N * D
    return np.float32((saa + sbb - 2.0 * sab) / denom)


def kernel(mu_a, logvar_a, mu_b, logvar_b):
    res = run_device(mu_a, logvar_a, mu_b, logvar_b, trace=False)
    return reduce_host(res.results)
